# revision 15
# baseline (speedup 1.0000x reference)
"""GraphSAGE (2-layer, mean aggr) + linear head on 8 trn2 NeuronCores.

Sharding: nodes partitioned across 8 cores (6272 each, padded to 50176).
Each core gathers the source features for ALL edges whose dst lands in its
node block (x replicated in HBM), aggregates them via one-hot matmuls on the
PE (segment-mean folded in), applies the dense SAGE layers, AllGathers h1 so
layer 2 can gather from the full table, and emits its block of (out, h).
"""

import numpy as np

import concourse.bacc as bacc
import concourse.bass as bass
import concourse.mybir as mybir
import concourse.tile as tile
from concourse.bass_utils import run_bass_kernel_spmd

N = 50000
D = 128
NCORES = 8
BLK = 6272            # nodes per core (NPAD / NCORES)
NPAD = NCORES * BLK   # 50176
TILES = BLK // 128    # 49 dst tiles of 128 nodes per core
HALF = NPAD // 2      # 25088 — int16 gather index limit forces a lo/hi table split
G = 4                 # dst tiles per gather group
PAD_SLOT = 200.0      # one-hot slot that never matches iota 0..127

f32 = mybir.dt.float32

TRACE = False        # test harness can flip this to get an NTFF profile
LAST_RESULT = None   # BassKernelResults of the most recent run
LAST_NC = None       # compiled Bass module of the most recent run
LAST_IN_MAPS = None  # per-core input maps of the most recent run
_NC_CACHE = {}       # kpad bytes -> compiled Bass module


def _prep(x, edge_index):
    """Host-side shard prep. Returns per-core input maps + program params."""
    src = edge_index[0].astype(np.int64)
    dst = edge_index[1].astype(np.int64)

    x_pad = np.zeros((NPAD, D), np.float32)
    x_pad[: x.shape[0]] = x

    deg = np.bincount(dst, minlength=NPAD).astype(np.float32)
    invdeg = (1.0 / np.maximum(deg, 1.0)).astype(np.float32)

    core = dst // BLK
    til = (dst % BLK) >> 7
    half = (src >= HALF).astype(np.int64)
    key = (core * TILES + til) * 2 + half
    order = np.argsort(key, kind="stable")
    src_s = src[order]
    dst_s = dst[order]
    counts = np.bincount(key, minlength=NCORES * TILES * 2).reshape(NCORES, TILES, 2)
    # SPMD: one program for all cores -> per-(tile,half) count = max over cores,
    # padded to a whole 128-edge chunk.
    kpad = ((counts.max(axis=0) + 127) // 128) * 128  # [TILES, 2]
    bounds = np.zeros(NCORES * TILES * 2 + 1, np.int64)
    np.cumsum(counts.reshape(-1), out=bounds[1:])

    nlo = kpad[:, 0] // 128
    nhi = kpad[:, 1] // 128
    tlo = int(kpad[:, 0].sum())
    thi = int(kpad[:, 1].sum())
    tt = tlo + thi
    groups = [list(range(s, min(s + G, TILES))) for s in range(0, TILES, G)]

    per_core = []
    for c in range(NCORES):
        ilo = np.zeros(tlo, np.int16)
        ihi = np.zeros(thi, np.int16)
        slo = np.full(tt, PAD_SLOT, np.float32)
        ivd = np.zeros(tt, np.float32)
        plo = phi = pc = 0
        for grp in groups:
            for t in grp:
                g = (c * TILES + t) * 2
                cnt = counts[c, t, 0]
                s0 = bounds[g]
                ilo[plo : plo + cnt] = src_s[s0 : s0 + cnt].astype(np.int16)
                d = dst_s[s0 : s0 + cnt]
                slo[pc : pc + cnt] = (d % 128).astype(np.float32)
                ivd[pc : pc + cnt] = invdeg[d]
                plo += kpad[t, 0]
                pc += kpad[t, 0]
            for t in grp:
                g = (c * TILES + t) * 2 + 1
                cnt = counts[c, t, 1]
                s0 = bounds[g]
                ihi[phi : phi + cnt] = (src_s[s0 : s0 + cnt] - HALF).astype(np.int16)
                d = dst_s[s0 : s0 + cnt]
                slo[pc : pc + cnt] = (d % 128).astype(np.float32)
                ivd[pc : pc + cnt] = invdeg[d]
                phi += kpad[t, 1]
                pc += kpad[t, 1]
        # idx tiles span 128 partitions; idx i lives at (i % 16, i // 16).
        # HW: for queue q the rx gpsimd CPU reads idxs from partitions
        # [32q, 32q+16) and the tx CPU from [32q+16, 32q+32) — replicate the
        # 16-row layout across all 128 partitions so any queue works.
        ilo128 = np.zeros((128, tlo // 16), np.int16)
        ilo128[:16] = ilo.reshape(-1, 16).T
        ilo128[:] = np.tile(ilo128[:16], (8, 1))
        ihi128 = np.zeros((128, thi // 16), np.int16)
        ihi128[:16] = ihi.reshape(-1, 16).T
        ihi128[:] = np.tile(ihi128[:16], (8, 1))
        per_core.append(
            dict(
                idx_lo=ilo128,
                idx_hi=ihi128,
                slots=np.ascontiguousarray(slo.reshape(-1, 128).T),
                invd=np.ascontiguousarray(ivd.reshape(-1, 128).T),
                xT=np.ascontiguousarray(x_pad[c * BLK : (c + 1) * BLK].T),
            )
        )
    return x_pad, per_core, kpad, groups


def _build(kpad, groups):
    """Build + compile the SPMD Bass program (parameterized by the padded
    per-(tile,half) chunk counts, identical on every core)."""
    nlo = kpad[:, 0] // 128
    nhi = kpad[:, 1] // 128
    nch = nlo + nhi
    tlo = int(kpad[:, 0].sum())
    thi = int(kpad[:, 1].sum())
    tt = tlo + thi

    nc = bacc.Bacc("TRN2", target_bir_lowering=False, debug=False,
                   num_devices=NCORES)

    x_pad = nc.dram_tensor("x_pad", [NPAD, D], f32, kind="ExternalInput")
    xT_in = nc.dram_tensor("xT", [D, BLK], f32, kind="ExternalInput")
    idx_lo_d = nc.dram_tensor("idx_lo", [128, tlo // 16], mybir.dt.int16,
                              kind="ExternalInput")
    idx_hi_d = nc.dram_tensor("idx_hi", [128, thi // 16], mybir.dt.int16,
                              kind="ExternalInput")
    slots_d = nc.dram_tensor("slots", [128, tt // 128], f32, kind="ExternalInput")
    invd_d = nc.dram_tensor("invd", [128, tt // 128], f32, kind="ExternalInput")
    wl1_d = nc.dram_tensor("W_l1", [D, D], f32, kind="ExternalInput")
    wr1_d = nc.dram_tensor("W_r1", [D, D], f32, kind="ExternalInput")
    wl2_d = nc.dram_tensor("W_l2", [D, D], f32, kind="ExternalInput")
    wr2_d = nc.dram_tensor("W_r2", [D, D], f32, kind="ExternalInput")
    bl1_d = nc.dram_tensor("b_l1", [1, D], f32, kind="ExternalInput")
    bl2_d = nc.dram_tensor("b_l2", [1, D], f32, kind="ExternalInput")
    wout_d = nc.dram_tensor("W_out", [D, 1], f32, kind="ExternalInput")
    bout_d = nc.dram_tensor("b_out", [1, 1], f32, kind="ExternalInput")
    iota_d = nc.dram_tensor("iota", [128, 128], f32, kind="ExternalInput")
    ones_d = nc.dram_tensor("ones", [1, 128], f32, kind="ExternalInput")

    h_out = nc.dram_tensor("h_out", [BLK, D], f32, kind="ExternalOutput")
    o_out = nc.dram_tensor("o_out", [1, BLK], f32, kind="ExternalOutput")

    h1_blk = nc.dram_tensor("h1_blk", [BLK, D], f32)
    h1_full = nc.dram_tensor("h1_full", [NPAD, D], f32, addr_space="Shared")

    relu = mybir.ActivationFunctionType.Relu
    copy = mybir.ActivationFunctionType.Copy
    iseq = mybir.AluOpType.is_equal
    mult = mybir.AluOpType.mult

    with tile.TileContext(nc) as tc:
        res_ctx = tc.tile_pool(name="resident", bufs=1)
        res_pool = res_ctx.__enter__()

        def _t(shape, dtype, name):
            return res_pool.tile(shape, dtype, name=name)

        # resident tensors
        xT = _t([D, BLK], f32, "xT_sb")
        h1T = _t([D, BLK], f32, "h1T_sb")
        idx_lo = _t([128, tlo // 16], mybir.dt.int16, "idx_lo_sb")
        idx_hi = _t([128, thi // 16], mybir.dt.int16, "idx_hi_sb")
        slots = _t([128, tt // 128], f32, "slots_sb")
        invd = _t([128, tt // 128], f32, "invd_sb")
        wl1 = _t([D, D], f32, "wl1_sb")
        wr1 = _t([D, D], f32, "wr1_sb")
        wl2 = _t([D, D], f32, "wl2_sb")
        wr2 = _t([D, D], f32, "wr2_sb")
        bl1 = _t([1, D], f32, "bl1_sb")
        bl2 = _t([1, D], f32, "bl2_sb")
        wout = _t([D, 1], f32, "wout_sb")
        bout = _t([1, 1], f32, "bout_sb")
        iota = _t([128, 128], f32, "iota_sb")
        ones = _t([1, 128], f32, "ones_sb")
        o_sb = _t([1, BLK], f32, "o_sb")

        for sb, dr in [(xT, xT_in), (idx_lo, idx_lo_d), (idx_hi, idx_hi_d),
                       (slots, slots_d), (invd, invd_d), (wl1, wl1_d),
                       (wr1, wr1_d), (wl2, wl2_d), (wr2, wr2_d), (bl1, bl1_d),
                       (bl2, bl2_d), (wout, wout_d), (bout, bout_d),
                       (iota, iota_d), (ones, ones_d)]:
            nc.sync.dma_start(sb[:], dr[:])

        max_grp_nch = max(int(nch[list(g)].sum()) for g in groups)

        with (
            tc.tile_pool(name="msg", bufs=2) as msg_pool,
            tc.tile_pool(name="onehot", bufs=4) as oh_pool,
            tc.tile_pool(name="meanT", bufs=2) as mt_pool,
            tc.tile_pool(name="hsb", bufs=2) as h_pool,
            tc.tile_pool(name="h2T", bufs=2) as h2t_pool,
            tc.tile_pool(name="psA", bufs=2, space="PSUM") as psA_pool,
            tc.tile_pool(name="psB", bufs=2, space="PSUM") as psB_pool,
            tc.tile_pool(name="psBT", bufs=2, space="PSUM") as psBT_pool,
            tc.tile_pool(name="psH", bufs=2, space="PSUM") as psH_pool,
        ):
            def layer(li, table, xT_src, wl, wr, bl):
                klo_base = 0  # running idx-column base (16-wide) for lo
                khi_base = 0
                cbase = 0     # running chunk (slot-column) base
                for grp in groups:
                    g_nlo = int(nlo[grp].sum())
                    g_nhi = int(nhi[grp].sum())
                    g_nch = g_nlo + g_nhi
                    msg = msg_pool.tile([128, max_grp_nch, 128], f32,
                                        name="msg")
                    # HW descriptor-ring limit: <=1024 idxs (8 chunks) per
                    # dma_gather — slice larger gathers.
                    for co in range(0, g_nlo, 8):
                        sc = min(8, g_nlo - co)
                        n = sc * 128
                        nc.gpsimd.dma_gather(
                            msg[:, co : co + sc, :], table[0:HALF, :],
                            idx_lo[:, klo_base + co * 8 :
                                   klo_base + co * 8 + n // 16],
                            n, n, D)
                    for co in range(0, g_nhi, 8):
                        sc = min(8, g_nhi - co)
                        n = sc * 128
                        nc.gpsimd.dma_gather(
                            msg[:, g_nlo + co : g_nlo + co + sc, :],
                            table[HALF:NPAD, :],
                            idx_hi[:, khi_base + co * 8 :
                                   khi_base + co * 8 + n // 16],
                            n, n, D)
                    klo_base += g_nlo * 8
                    khi_base += g_nhi * 8

                    lo_off = 0
                    hi_off = 0
                    for t in grp:
                        cols = [lo_off + j for j in range(int(nlo[t]))] + \
                               [g_nlo + hi_off + j for j in range(int(nhi[t]))]
                        lo_off += int(nlo[t])
                        hi_off += int(nhi[t])

                        psA = psA_pool.tile([128, 128], f32, name="psA")
                        for i, mcol in enumerate(cols):
                            oh = oh_pool.tile([128, 128], f32, name="oh")
                            scol = cbase + mcol
                            nc.vector.tensor_scalar(
                                oh[:], iota[:],
                                slots[:, scol : scol + 1],
                                invd[:, scol : scol + 1],
                                iseq, mult)
                            nc.tensor.matmul(
                                psA[:], msg[:, mcol, :], oh[:],
                                start=(i == 0), stop=(i == len(cols) - 1))
                        meanT = mt_pool.tile([128, 128], f32, name="meanT")
                        nc.scalar.activation(meanT[:], psA[:], copy)

                        xt_t = xT_src[:, t * 128 : (t + 1) * 128]
                        psB = psB_pool.tile([128, 128], f32, name="psB")
                        nc.tensor.matmul(psB[:], meanT[:], wl[:], start=True,
                                         stop=False)
                        nc.tensor.matmul(psB[:], xt_t, wr[:], start=False,
                                         stop=False)
                        nc.tensor.matmul(psB[:], ones[:], bl[:], start=False,
                                         stop=True)
                        psBT = psBT_pool.tile([128, 128], f32, name="psBT")
                        nc.tensor.matmul(psBT[:], wl[:], meanT[:], start=True,
                                         stop=False)
                        nc.tensor.matmul(psBT[:], wr[:], xt_t, start=False,
                                         stop=False)
                        nc.tensor.matmul(psBT[:], bl[:], ones[:], start=False,
                                         stop=True)

                        h_sb = h_pool.tile([128, 128], f32, name="hsb")
                        nc.scalar.activation(h_sb[:], psB[:], relu)
                        if li == 1:
                            nc.scalar.activation(
                                h1T[:, t * 128 : (t + 1) * 128], psBT[:], relu)
                            nc.sync.dma_start(
                                h1_blk[t * 128 : (t + 1) * 128, :], h_sb[:])
                        else:
                            h2T = h2t_pool.tile([128, 128], f32, name="h2T")
                            nc.scalar.activation(h2T[:], psBT[:], relu)
                            nc.sync.dma_start(
                                h_out[t * 128 : (t + 1) * 128, :], h_sb[:])
                            psH = psH_pool.tile([128, 128], f32, name="psH")
                            nc.tensor.matmul(psH[0:1, :], wout[:], h2T[:],
                                             start=True, stop=False)
                            nc.tensor.matmul(psH[0:1, :], bout[:], ones[:],
                                             start=False, stop=True)
                            nc.scalar.activation(
                                o_sb[0:1, t * 128 : (t + 1) * 128],
                                psH[0:1, :], copy)
                    cbase += g_nch

            layer(1, x_pad, xT, wl1, wr1, bl1)
            nc.gpsimd.collective_compute(
                "AllGather", mybir.AluOpType.bypass,
                replica_groups=[list(range(NCORES))],
                ins=[h1_blk[:]], outs=[h1_full[:]])
            layer(2, h1_full, h1T, wl2, wr2, bl2)
            nc.sync.dma_start(o_out[:], o_sb[:])
        res_ctx.__exit__(None, None, None)

    nc.compile()
    return nc


def kernel(**inputs):
    x = np.asarray(inputs["x"], np.float32)
    edge_index = np.asarray(inputs["edge_index"])

    x_pad, per_core, kpad, groups = _prep(x, edge_index)
    ck = kpad.tobytes()
    nc = _NC_CACHE.get(ck)
    if nc is None:
        nc = _NC_CACHE[ck] = _build(kpad, groups)

    common = dict(
        x_pad=x_pad,
        W_l1=np.asarray(inputs["W_l1"], np.float32),
        W_r1=np.asarray(inputs["W_r1"], np.float32),
        W_l2=np.asarray(inputs["W_l2"], np.float32),
        W_r2=np.asarray(inputs["W_r2"], np.float32),
        b_l1=np.asarray(inputs["b_l1"], np.float32).reshape(1, D),
        b_l2=np.asarray(inputs["b_l2"], np.float32).reshape(1, D),
        W_out=np.asarray(inputs["W_out"], np.float32).reshape(D, 1),
        b_out=np.asarray(inputs["b_out"], np.float32).reshape(1, 1),
        iota=np.ascontiguousarray(
            np.tile(np.arange(128, dtype=np.float32), (128, 1))),
        ones=np.ones((1, 128), np.float32),
    )
    in_maps = [{**common, **pc} for pc in per_core]

    res = run_bass_kernel_spmd(nc, in_maps, list(range(NCORES)), trace=TRACE)
    global LAST_RESULT, LAST_NC, LAST_IN_MAPS
    LAST_RESULT = res
    LAST_NC = nc
    LAST_IN_MAPS = in_maps
    h = np.concatenate([r["h_out"] for r in res.results], axis=0)[:N]
    o = np.concatenate([r["o_out"][0] for r in res.results], axis=0)[:N]
    return o, h


# revision 18
# speedup vs baseline: 1.0585x; 1.0585x over previous
"""GraphSAGE (2-layer, mean aggr) + linear head on 8 trn2 NeuronCores.

Sharding: nodes partitioned across 8 cores (6272 each, padded to 50176).
Each core gathers the source features for ALL edges whose dst lands in its
node block (x replicated in HBM), aggregates them via one-hot matmuls on the
PE (segment-mean folded in), applies the dense SAGE layers, AllGathers h1 so
layer 2 can gather from the full table, and emits its block of (out, h).
"""

import numpy as np

import concourse.bacc as bacc
import concourse.bass as bass
import concourse.mybir as mybir
import concourse.tile as tile
from concourse.bass_utils import run_bass_kernel_spmd

N = 50000
D = 128
NCORES = 8
BLK = 6272            # nodes per core (NPAD / NCORES)
NPAD = NCORES * BLK   # 50176
TILES = BLK // 128    # 49 dst tiles of 128 nodes per core
HALF = NPAD // 2      # 25088 — int16 gather index limit forces a lo/hi table split
G = 4                 # dst tiles per gather group
PAD_SLOT = 200.0      # one-hot slot that never matches iota 0..127

f32 = mybir.dt.float32

TRACE = False        # test harness can flip this to get an NTFF profile
LAST_RESULT = None   # BassKernelResults of the most recent run
LAST_NC = None       # compiled Bass module of the most recent run
LAST_IN_MAPS = None  # per-core input maps of the most recent run
_NC_CACHE = {}       # kpad bytes -> compiled Bass module


def _prep(x, edge_index):
    """Host-side shard prep. Returns per-core input maps + program params."""
    src = edge_index[0].astype(np.int64)
    dst = edge_index[1].astype(np.int64)

    x_pad = np.zeros((NPAD, D), np.float32)
    x_pad[: x.shape[0]] = x

    deg = np.bincount(dst, minlength=NPAD).astype(np.float32)
    invdeg = (1.0 / np.maximum(deg, 1.0)).astype(np.float32)

    core = dst // BLK
    til = (dst % BLK) >> 7
    half = (src >= HALF).astype(np.int64)
    key = (core * TILES + til) * 2 + half
    order = np.argsort(key, kind="stable")
    src_s = src[order]
    dst_s = dst[order]
    counts = np.bincount(key, minlength=NCORES * TILES * 2).reshape(NCORES, TILES, 2)
    # SPMD: one program for all cores -> per-(tile,half) count = max over cores,
    # padded to a whole 128-edge chunk.
    kpad = ((counts.max(axis=0) + 127) // 128) * 128  # [TILES, 2]
    bounds = np.zeros(NCORES * TILES * 2 + 1, np.int64)
    np.cumsum(counts.reshape(-1), out=bounds[1:])

    nlo = kpad[:, 0] // 128
    nhi = kpad[:, 1] // 128
    tlo = int(kpad[:, 0].sum())
    thi = int(kpad[:, 1].sum())
    tt = tlo + thi
    groups = [list(range(s, min(s + G, TILES))) for s in range(0, TILES, G)]

    per_core = []
    for c in range(NCORES):
        ilo = np.zeros(tlo, np.int16)
        ihi = np.zeros(thi, np.int16)
        slo = np.full(tt, PAD_SLOT, np.float32)
        ivd = np.zeros(tt, np.float32)
        plo = phi = pc = 0
        for grp in groups:
            for t in grp:
                g = (c * TILES + t) * 2
                cnt = counts[c, t, 0]
                s0 = bounds[g]
                ilo[plo : plo + cnt] = src_s[s0 : s0 + cnt].astype(np.int16)
                d = dst_s[s0 : s0 + cnt]
                slo[pc : pc + cnt] = (d % 128).astype(np.float32)
                ivd[pc : pc + cnt] = invdeg[d]
                plo += kpad[t, 0]
                pc += kpad[t, 0]
            for t in grp:
                g = (c * TILES + t) * 2 + 1
                cnt = counts[c, t, 1]
                s0 = bounds[g]
                ihi[phi : phi + cnt] = (src_s[s0 : s0 + cnt] - HALF).astype(np.int16)
                d = dst_s[s0 : s0 + cnt]
                slo[pc : pc + cnt] = (d % 128).astype(np.float32)
                ivd[pc : pc + cnt] = invdeg[d]
                phi += kpad[t, 1]
                pc += kpad[t, 1]
        # idx tiles span 128 partitions; idx i lives at (i % 16, i // 16).
        # HW: for queue q the rx gpsimd CPU reads idxs from partitions
        # [32q, 32q+16) and the tx CPU from [32q+16, 32q+32) — replicate the
        # 16-row layout across all 128 partitions so any queue works.
        ilo128 = np.zeros((128, tlo // 16), np.int16)
        ilo128[:16] = ilo.reshape(-1, 16).T
        ilo128[:] = np.tile(ilo128[:16], (8, 1))
        ihi128 = np.zeros((128, thi // 16), np.int16)
        ihi128[:16] = ihi.reshape(-1, 16).T
        ihi128[:] = np.tile(ihi128[:16], (8, 1))
        per_core.append(
            dict(
                idx_lo=ilo128,
                idx_hi=ihi128,
                slots=np.ascontiguousarray(slo.reshape(-1, 128).T),
                invd=np.ascontiguousarray(ivd.reshape(-1, 128).T),
                xT=np.ascontiguousarray(x_pad[c * BLK : (c + 1) * BLK].T),
            )
        )
    return x_pad, per_core, kpad, groups


def _build(kpad, groups):
    """Build + compile the SPMD Bass program (parameterized by the padded
    per-(tile,half) chunk counts, identical on every core)."""
    nlo = kpad[:, 0] // 128
    nhi = kpad[:, 1] // 128
    nch = nlo + nhi
    tlo = int(kpad[:, 0].sum())
    thi = int(kpad[:, 1].sum())
    tt = tlo + thi

    nc = bacc.Bacc("TRN2", target_bir_lowering=False, debug=False,
                   num_devices=NCORES, num_swdge_queues=4)

    x_pad = nc.dram_tensor("x_pad", [NPAD, D], f32, kind="ExternalInput")
    xT_in = nc.dram_tensor("xT", [D, BLK], f32, kind="ExternalInput")
    idx_lo_d = nc.dram_tensor("idx_lo", [128, tlo // 16], mybir.dt.int16,
                              kind="ExternalInput")
    idx_hi_d = nc.dram_tensor("idx_hi", [128, thi // 16], mybir.dt.int16,
                              kind="ExternalInput")
    slots_d = nc.dram_tensor("slots", [128, tt // 128], f32, kind="ExternalInput")
    invd_d = nc.dram_tensor("invd", [128, tt // 128], f32, kind="ExternalInput")
    wl1_d = nc.dram_tensor("W_l1", [D, D], f32, kind="ExternalInput")
    wr1_d = nc.dram_tensor("W_r1", [D, D], f32, kind="ExternalInput")
    wl2_d = nc.dram_tensor("W_l2", [D, D], f32, kind="ExternalInput")
    wr2_d = nc.dram_tensor("W_r2", [D, D], f32, kind="ExternalInput")
    bl1_d = nc.dram_tensor("b_l1", [1, D], f32, kind="ExternalInput")
    bl2_d = nc.dram_tensor("b_l2", [1, D], f32, kind="ExternalInput")
    wout_d = nc.dram_tensor("W_out", [D, 1], f32, kind="ExternalInput")
    bout_d = nc.dram_tensor("b_out", [1, 1], f32, kind="ExternalInput")
    iota_d = nc.dram_tensor("iota", [128, 128], f32, kind="ExternalInput")
    ones_d = nc.dram_tensor("ones", [1, 128], f32, kind="ExternalInput")

    h_out = nc.dram_tensor("h_out", [BLK, D], f32, kind="ExternalOutput")
    o_out = nc.dram_tensor("o_out", [1, BLK], f32, kind="ExternalOutput")

    h1_blk = nc.dram_tensor("h1_blk", [BLK, D], f32)
    h1_full = nc.dram_tensor("h1_full", [NPAD, D], f32, addr_space="Shared")

    relu = mybir.ActivationFunctionType.Relu
    copy = mybir.ActivationFunctionType.Copy
    iseq = mybir.AluOpType.is_equal
    mult = mybir.AluOpType.mult

    with tile.TileContext(nc) as tc:
        res_ctx = tc.tile_pool(name="resident", bufs=1)
        res_pool = res_ctx.__enter__()

        def _t(shape, dtype, name):
            return res_pool.tile(shape, dtype, name=name)

        # resident tensors
        xT = _t([D, BLK], f32, "xT_sb")
        h1T = _t([D, BLK], f32, "h1T_sb")
        idx_lo = _t([128, tlo // 16], mybir.dt.int16, "idx_lo_sb")
        idx_hi = _t([128, thi // 16], mybir.dt.int16, "idx_hi_sb")
        slots = _t([128, tt // 128], f32, "slots_sb")
        invd = _t([128, tt // 128], f32, "invd_sb")
        wl1 = _t([D, D], f32, "wl1_sb")
        wr1 = _t([D, D], f32, "wr1_sb")
        wl2 = _t([D, D], f32, "wl2_sb")
        wr2 = _t([D, D], f32, "wr2_sb")
        bl1 = _t([1, D], f32, "bl1_sb")
        bl2 = _t([1, D], f32, "bl2_sb")
        wout = _t([D, 1], f32, "wout_sb")
        bout = _t([1, 1], f32, "bout_sb")
        iota = _t([128, 128], f32, "iota_sb")
        ones = _t([1, 128], f32, "ones_sb")
        o_sb = _t([1, BLK], f32, "o_sb")

        for sb, dr in [(xT, xT_in), (idx_lo, idx_lo_d), (idx_hi, idx_hi_d),
                       (slots, slots_d), (invd, invd_d), (wl1, wl1_d),
                       (wr1, wr1_d), (wl2, wl2_d), (wr2, wr2_d), (bl1, bl1_d),
                       (bl2, bl2_d), (wout, wout_d), (bout, bout_d),
                       (iota, iota_d), (ones, ones_d)]:
            nc.sync.dma_start(sb[:], dr[:])

        max_grp_nch = max(int(nch[list(g)].sum()) for g in groups)

        with (
            tc.tile_pool(name="msg", bufs=2) as msg_pool,
            tc.tile_pool(name="onehot", bufs=4) as oh_pool,
            tc.tile_pool(name="meanT", bufs=2) as mt_pool,
            tc.tile_pool(name="hsb", bufs=2) as h_pool,
            tc.tile_pool(name="h2T", bufs=2) as h2t_pool,
            tc.tile_pool(name="psA", bufs=2, space="PSUM") as psA_pool,
            tc.tile_pool(name="psB", bufs=2, space="PSUM") as psB_pool,
            tc.tile_pool(name="psBT", bufs=2, space="PSUM") as psBT_pool,
            tc.tile_pool(name="psH", bufs=2, space="PSUM") as psH_pool,
        ):
            def layer(li, table, xT_src, wl, wr, bl):
                klo_base = 0  # running idx-column base (16-wide) for lo
                khi_base = 0
                cbase = 0     # running chunk (slot-column) base
                qctr = 0      # round-robin over the 4 SWDGE queues
                for grp in groups:
                    g_nlo = int(nlo[grp].sum())
                    g_nhi = int(nhi[grp].sum())
                    g_nch = g_nlo + g_nhi
                    msg = msg_pool.tile([128, max_grp_nch, 128], f32,
                                        name="msg")
                    # HW descriptor-ring limit: <=1024 idxs (8 chunks) per
                    # dma_gather — slice larger gathers.
                    for co in range(0, g_nlo, 8):
                        sc = min(8, g_nlo - co)
                        n = sc * 128
                        nc.gpsimd.dma_gather(
                            msg[:, co : co + sc, :], table[0:HALF, :],
                            idx_lo[:, klo_base + co * 8 :
                                   klo_base + co * 8 + n // 16],
                            n, n, D, queue_num=qctr % 4)
                        qctr += 1
                    for co in range(0, g_nhi, 8):
                        sc = min(8, g_nhi - co)
                        n = sc * 128
                        nc.gpsimd.dma_gather(
                            msg[:, g_nlo + co : g_nlo + co + sc, :],
                            table[HALF:NPAD, :],
                            idx_hi[:, khi_base + co * 8 :
                                   khi_base + co * 8 + n // 16],
                            n, n, D, queue_num=qctr % 4)
                        qctr += 1
                    klo_base += g_nlo * 8
                    khi_base += g_nhi * 8

                    lo_off = 0
                    hi_off = 0
                    for t in grp:
                        cols = [lo_off + j for j in range(int(nlo[t]))] + \
                               [g_nlo + hi_off + j for j in range(int(nhi[t]))]
                        lo_off += int(nlo[t])
                        hi_off += int(nhi[t])

                        psA = psA_pool.tile([128, 128], f32, name="psA")
                        for i, mcol in enumerate(cols):
                            oh = oh_pool.tile([128, 128], f32, name="oh")
                            scol = cbase + mcol
                            nc.vector.tensor_scalar(
                                oh[:], iota[:],
                                slots[:, scol : scol + 1],
                                invd[:, scol : scol + 1],
                                iseq, mult)
                            nc.tensor.matmul(
                                psA[:], msg[:, mcol, :], oh[:],
                                start=(i == 0), stop=(i == len(cols) - 1))
                        meanT = mt_pool.tile([128, 128], f32, name="meanT")
                        nc.scalar.activation(meanT[:], psA[:], copy)

                        xt_t = xT_src[:, t * 128 : (t + 1) * 128]
                        psB = psB_pool.tile([128, 128], f32, name="psB")
                        nc.tensor.matmul(psB[:], meanT[:], wl[:], start=True,
                                         stop=False)
                        nc.tensor.matmul(psB[:], xt_t, wr[:], start=False,
                                         stop=False)
                        nc.tensor.matmul(psB[:], ones[:], bl[:], start=False,
                                         stop=True)
                        psBT = psBT_pool.tile([128, 128], f32, name="psBT")
                        nc.tensor.matmul(psBT[:], wl[:], meanT[:], start=True,
                                         stop=False)
                        nc.tensor.matmul(psBT[:], wr[:], xt_t, start=False,
                                         stop=False)
                        nc.tensor.matmul(psBT[:], bl[:], ones[:], start=False,
                                         stop=True)

                        h_sb = h_pool.tile([128, 128], f32, name="hsb")
                        nc.scalar.activation(h_sb[:], psB[:], relu)
                        if li == 1:
                            nc.scalar.activation(
                                h1T[:, t * 128 : (t + 1) * 128], psBT[:], relu)
                            nc.sync.dma_start(
                                h1_blk[t * 128 : (t + 1) * 128, :], h_sb[:])
                        else:
                            h2T = h2t_pool.tile([128, 128], f32, name="h2T")
                            nc.scalar.activation(h2T[:], psBT[:], relu)
                            nc.sync.dma_start(
                                h_out[t * 128 : (t + 1) * 128, :], h_sb[:])
                            psH = psH_pool.tile([128, 128], f32, name="psH")
                            nc.tensor.matmul(psH[0:1, :], wout[:], h2T[:],
                                             start=True, stop=False)
                            nc.tensor.matmul(psH[0:1, :], bout[:], ones[:],
                                             start=False, stop=True)
                            nc.scalar.activation(
                                o_sb[0:1, t * 128 : (t + 1) * 128],
                                psH[0:1, :], copy)
                    cbase += g_nch

            layer(1, x_pad, xT, wl1, wr1, bl1)
            nc.gpsimd.collective_compute(
                "AllGather", mybir.AluOpType.bypass,
                replica_groups=[list(range(NCORES))],
                ins=[h1_blk[:]], outs=[h1_full[:]])
            layer(2, h1_full, h1T, wl2, wr2, bl2)
            nc.sync.dma_start(o_out[:], o_sb[:])
        res_ctx.__exit__(None, None, None)

    nc.compile()
    return nc


def kernel(**inputs):
    x = np.asarray(inputs["x"], np.float32)
    edge_index = np.asarray(inputs["edge_index"])

    x_pad, per_core, kpad, groups = _prep(x, edge_index)
    ck = kpad.tobytes()
    nc = _NC_CACHE.get(ck)
    if nc is None:
        nc = _NC_CACHE[ck] = _build(kpad, groups)

    common = dict(
        x_pad=x_pad,
        W_l1=np.asarray(inputs["W_l1"], np.float32),
        W_r1=np.asarray(inputs["W_r1"], np.float32),
        W_l2=np.asarray(inputs["W_l2"], np.float32),
        W_r2=np.asarray(inputs["W_r2"], np.float32),
        b_l1=np.asarray(inputs["b_l1"], np.float32).reshape(1, D),
        b_l2=np.asarray(inputs["b_l2"], np.float32).reshape(1, D),
        W_out=np.asarray(inputs["W_out"], np.float32).reshape(D, 1),
        b_out=np.asarray(inputs["b_out"], np.float32).reshape(1, 1),
        iota=np.ascontiguousarray(
            np.tile(np.arange(128, dtype=np.float32), (128, 1))),
        ones=np.ones((1, 128), np.float32),
    )
    in_maps = [{**common, **pc} for pc in per_core]

    res = run_bass_kernel_spmd(nc, in_maps, list(range(NCORES)), trace=TRACE)
    global LAST_RESULT, LAST_NC, LAST_IN_MAPS
    LAST_RESULT = res
    LAST_NC = nc
    LAST_IN_MAPS = in_maps
    h = np.concatenate([r["h_out"] for r in res.results], axis=0)[:N]
    o = np.concatenate([r["o_out"][0] for r in res.results], axis=0)[:N]
    return o, h
